# revision 4
# baseline (speedup 1.0000x reference)
"""CRF loss kernel for Trainium2 (Bass/Tile), 8-core data parallel.

Math (per batch row b):
  llh[b] = score[b] - logZ[b];  output = mean_b llh[b]

Denominator (logZ) on device via the *scaled linear-space* forward algorithm:
  alpha recursion in log space == p recursion in linear space:
      p_t = (expT^T @ p_{t-1}) * e_t        e_t = exp(emissions[:, t, :])
  with a constant per-step rescale e^{-C0} folded into the stationary
  expT_s = exp(T - C0) so values stay in f32/bf16 range.  The sequence is
  split fwd (t=0..T/2-1) and bwd (t=T-1..T/2, beta recursion), halving the
  serial depth; both chains are stacked on partitions (rows 0-32 fwd,
  64-96 bwd) so one concurrent quadrant-matmul pair + one DVE multiply
  advances both.
  logZ[b] = log( p_{T/2-1}^T expT_s m_{T/2} ) + (T-1)*C0

Layout: emissions are uploaded PRE-TRANSPOSED on the host in k-major form
[K, T, B] (bf16), so every e_t tile lands in SBUF via a plain contiguous
DMA in exactly the [k(partition), b(free)] layout the scan needs (v1 spent
622us on 512 on-device DMA-xbar transposes).  The batch columns are split
into 2 independent chains (128 cols each) that ping-pong between TensorE
and VectorE, hiding matmul + semaphore latency behind DVE throughput (the
scan's floor: 2x(120+128) DVE cycles per round).

The tiny O(K^2) constants (exp(T - C0) stationary, exp(start/end)) are
precomputed host-side and DMA'd in, which keeps the device prologue to a
few us (v2 spent 15us on memsets + setup ACTs before round 1).

Numerator: score[b] = sum_t em[b,t,tag[b,t]] + transitions along the tag
path + start/end terms.  Pure index arithmetic (0.05% of FLOPs, O(B*T)
gathers), computed host-side in f64 like the baseline already did for the
transition part.  All O(B*T*K) math runs on device.

Sharding: pure data parallel over batch (2048 -> 8 cores x 256), small
tensors replicated; per-core partial outputs are combined on host.
"""

from contextlib import ExitStack

import numpy as np

import concourse.bass as bass
import concourse.bacc as bacc
import concourse.tile as tile
from concourse import mybir
from concourse.bass_utils import run_bass_kernel_spmd

import ml_dtypes

BF16 = ml_dtypes.bfloat16

F32 = mybir.dt.float32
BF = mybir.dt.bfloat16

# Problem constants
B_FULL, T_FULL, K = 2048, 512, 33
N_CORES = 8
BC = B_FULL // N_CORES  # 256 batch rows per core
NB = BC                 # batch columns per core in SBUF
NS = T_FULL // 2        # slots per direction (fwd / bwd chains)
C0 = 3.9832  # per-step log-growth rescale (see module docstring)


def build_crf_module(n_chains=2, W=16, prefetch=2):
    """Per-core Bass module computing scaled Z (denominator) only."""
    NBLK = NS // W
    assert NS % W == 0
    assert NB % n_chains == 0
    CW = NB // n_chains  # columns per chain

    nc = bacc.Bacc()

    # ---- DRAM I/O (per-core shapes) ----
    # k-major emissions, host pre-transposed: [K, NS, NB]; slot s of emf is
    # t=s, slot s of emb is t=T-1-s (host pre-reversed).
    emf_d = nc.dram_tensor("emf", [K, NS, NB], BF, kind="ExternalInput")
    emb_d = nc.dram_tensor("emb", [K, NS, NB], BF, kind="ExternalInput")
    # host-precomputed constants (O(K^2) setup work):
    # expt: [128, 64] bf16, rows 0:33 = exp(T - C0), rows 64:97 = exp(T^T - C0),
    #       cols 33:64 and all other rows zero.
    expt_d = nc.dram_tensor("expt", [128, 64], BF, kind="ExternalInput")
    # esev: [128] f32, rows 0:33 exp(start), 64:97 exp(end), rest 1.0
    ese_d = nc.dram_tensor("esev", [128], F32, kind="ExternalInput")
    zero_d = nc.dram_tensor("zerov", [1], F32, kind="ExternalInput")
    one_d = nc.dram_tensor("onev", [1], BF, kind="ExternalInput")
    zs_o = nc.dram_tensor("zs_o", [128, 2], F32, kind="ExternalOutput")

    with tile.TileContext(nc) as tc, ExitStack() as ctx:
        singles = ctx.enter_context(tc.tile_pool(name="singles", bufs=1))
        q_pool = ctx.enter_context(tc.tile_pool(name="q", bufs=2, space="PSUM"))
        z_pool = ctx.enter_context(tc.tile_pool(name="z", bufs=1, space="PSUM"))

        # ---------------- setup (all via DMA; no memset/ACT prologue) ----
        zero_c = singles.tile([128, 1], F32, tag="zero_c")
        nc.sync.dma_start(
            out=zero_c[:, :],
            in_=bass.AP(tensor=zero_d, offset=0, ap=[[0, 128], [1, 1]]),
        )
        expT = singles.tile([128, 64], BF, tag="expT")
        nc.sync.dma_start(out=expT[:, :], in_=expt_d[:, :])
        ese = singles.tile([128, 1], F32, tag="ese")
        nc.sync.dma_start(out=ese[:, :], in_=ese_d[:])
        ones_col = singles.tile([128, 1], BF, tag="ones_col")
        nc.sync.dma_start(
            out=ones_col[:, :],
            in_=bass.AP(tensor=one_d, offset=0, ap=[[0, 128], [1, 1]]),
        )

        # dummy first ACT op: forces the exp table-set load (~2.7us) to
        # overlap the first block DMA instead of gating round 1.
        actwarm = singles.tile([128, 1], F32, tag="actwarm")
        nc.scalar.activation(
            actwarm[:, :],
            zero_c[:, :],
            mybir.ActivationFunctionType.Exp,
            bias=zero_c[:, :],
        )

        # persistent block tiles.  raw rows 32:64 are zeroed once (row 32 is
        # re-written by every block DMA; exp(0)=1 on 33:64 keeps the
        # recursion well-defined); the DMA writes rows 0:33 / 64:97.
        NRAW = 2
        NET = 3
        raw_bufs = [
            singles.tile([128, W * NB], BF, tag=f"raw_{p}", name=f"raw_{p}")
            for p in range(NRAW)
        ]
        for p in range(NRAW):
            nc.vector.memset(raw_bufs[p][32:64, :], 0.0)
        eT_bufs = [
            singles.tile([128, W * NB], BF, tag=f"eT_{p}", name=f"eT_{p}")
            for p in range(NET)
        ]

        # persistent state ping-pong tiles (chains = column ranges)
        st = [
            singles.tile([128, NB], BF, tag=f"st_{p}", name=f"st_{p}")
            for p in range(2)
        ]

        def load_block(j, exp_chunks=1):
            """DMA raw emissions block j (fwd rows 0:33, bwd rows 64:97),
            then exp into the eT buffer (optionally in chunks so the first
            slots become available sooner)."""
            raw = raw_bufs[j % NRAW]
            nc.sync.dma_start(
                out=raw[0:K, :],
                in_=bass.AP(
                    tensor=emf_d, offset=j * W * NB, ap=[[NS * NB, K], [1, W * NB]]
                ),
            )
            nc.sync.dma_start(
                out=raw[64 : 64 + K, :],
                in_=bass.AP(
                    tensor=emb_d, offset=j * W * NB, ap=[[NS * NB, K], [1, W * NB]]
                ),
            )
            eT = eT_bufs[j % NET]
            step = W * NB // exp_chunks
            for i in range(exp_chunks):
                nc.scalar.activation(
                    eT[0:97, i * step : (i + 1) * step],
                    raw[0:97, i * step : (i + 1) * step],
                    mybir.ActivationFunctionType.Exp,
                    bias=zero_c[0:97, :],
                )
            return eT

        # ---------------- pipeline ----------------
        load_block(0, exp_chunks=4)
        for j in range(1, min(prefetch, NBLK)):
            load_block(j)

        eT_cur = None
        for s in range(NS):
            j, ls = divmod(s, W)
            if ls == 0:
                eT_cur = eT_bufs[j % NET]
                if j + prefetch < NBLK:
                    load_block(j + prefetch)
            if s == 0:
                # init: state = e0_stacked * exp(start/end) per-partition
                nc.vector.tensor_scalar(
                    out=st[0][0:97, :],
                    in0=eT_cur[0:97, 0:NB],
                    scalar1=ese[0:97, :],
                    scalar2=None,
                    op0=mybir.AluOpType.mult,
                )
                continue
            p = (s - 1) % 2
            for c in range(n_chains):
                lo = c * CW
                q = q_pool.tile([128, CW], F32, tag=f"q{c}")
                # concurrent quadrant pair: fwd (0,0), bwd (64,64); the
                # zero cols 33:64 of each stationary zero q rows 33:64 /
                # 97:128 so the TT below reads a fully-defined [0:97].
                nc.tensor.matmul(
                    out=q[0:64, :],
                    lhsT=expT[0:K, :],
                    rhs=st[p][0:K, lo : lo + CW],
                    start=True,
                    stop=True,
                    tile_position=(0, 0),
                )
                nc.tensor.matmul(
                    out=q[64:128, :],
                    lhsT=expT[64 : 64 + K, :],
                    rhs=st[p][64 : 64 + K, lo : lo + CW],
                    start=True,
                    stop=True,
                    tile_position=(64, 64),
                )
                nc.vector.tensor_tensor(
                    st[1 - p][0:97, lo : lo + CW],
                    q[0:97, :],
                    eT_cur[0:97, ls * NB + lo : ls * NB + lo + CW],
                    mybir.AluOpType.mult,
                )

        # ---------------- tail: combine fwd and bwd ----------------
        pfin = (NS - 1) % 2  # st[pfin] holds p_{NS-1} (rows 0:33), m_{NS-1} (64:97)
        beta = z_pool.tile([128, NB], F32, tag="beta")
        # beta = expT_s @ m ; bwd stationary block, output partition-aligned
        # with the fwd state (tile_position (64, 0)).
        nc.tensor.matmul(
            out=beta[0:64, :],
            lhsT=expT[64 : 64 + K, :],
            rhs=st[pfin][64 : 64 + K, :],
            start=True,
            stop=True,
            tile_position=(64, 0),
        )
        u = singles.tile([128, NB], BF, tag="u")
        nc.vector.tensor_tensor(
            u[0:K, :], beta[0:K, :], st[pfin][0:K, :], mybir.AluOpType.mult
        )
        # per-b partition sum via transposed ones-matmul (u.T @ ones) so the
        # result is partition-major; host takes log of the two columns.
        zt = z_pool.tile([128, 2], F32, tag="zt")
        for c in range(2):
            nc.tensor.matmul(
                out=zt[:, c : c + 1],
                lhsT=u[0:K, c * 128 : (c + 1) * 128],
                rhs=ones_col[0:K, :],
                start=True,
                stop=True,
                tile_position=(0, 0),
            )
        zs = singles.tile([128, 2], F32, tag="zs")
        nc.vector.tensor_copy(zs[:, :], zt[:, :])
        nc.sync.dma_start(
            out=bass.AP(tensor=zs_o, offset=0, ap=[[2, 128], [1, 2]]),
            in_=zs[:, :],
        )

    nc.finalize()
    return nc


_CACHE = {}
LAST_RESULT = None


def _get_module():
    key = "v3"
    if key not in _CACHE:
        _CACHE[key] = build_crf_module()
    return _CACHE[key]


def _host_reference(emissions, tags, mask, start_transitions, end_transitions, transitions):
    """Pure-numpy fallback (unused for the all-ones mask the spec generates)."""
    em = emissions.astype(np.float64)
    mk = mask.astype(np.float64)
    B, T, K_ = em.shape
    b_idx = np.arange(B)
    tg = tags.astype(np.int64)
    score = start_transitions[tg[:, 0]].astype(np.float64) + em[b_idx, 0, tg[:, 0]]
    prev = tg[:, 0]
    for t in range(1, T):
        step = transitions[prev, tg[:, t]] + em[b_idx, t, tg[:, t]]
        score = score + step * mk[:, t]
        prev = np.where(mk[:, t] > 0, tg[:, t], prev)
    score = score + end_transitions[prev]

    def lse(x, axis):
        m = x.max(axis=axis, keepdims=True)
        return (m + np.log(np.exp(x - m).sum(axis=axis, keepdims=True))).squeeze(axis)

    alpha = start_transitions[None, :] + em[:, 0, :]
    for t in range(1, T):
        nxt = lse(alpha[:, :, None] + transitions[None, :, :].astype(np.float64) + em[:, t, None, :], axis=1)
        alpha = np.where(mk[:, t][:, None] > 0, nxt, alpha)
    logZ = lse(alpha + end_transitions[None, :], axis=1)
    return np.float32((score - logZ).mean())


def kernel(emissions, tags, mask, start_transitions, end_transitions, transitions):
    emissions = np.asarray(emissions, dtype=np.float32)
    tags_i = np.asarray(tags).astype(np.int64)
    mask_np = np.asarray(mask)
    start_np = np.asarray(start_transitions, dtype=np.float32)
    end_np = np.asarray(end_transitions, dtype=np.float32)
    trans_np = np.asarray(transitions, dtype=np.float32)

    if not mask_np.all():
        return _host_reference(
            emissions, tags_i, mask_np, start_np, end_np, trans_np
        )

    nc = _get_module()

    # host-precomputed O(K^2) constants
    expt = np.zeros((128, 64), dtype=np.float64)
    expt[0:K, 0:K] = np.exp(trans_np.astype(np.float64) - C0)
    expt[64 : 64 + K, 0:K] = np.exp(trans_np.T.astype(np.float64) - C0)
    expt = expt.astype(BF16)
    esev = np.ones(128, dtype=np.float64)
    esev[0:K] = np.exp(start_np.astype(np.float64))
    esev[64 : 64 + K] = np.exp(end_np.astype(np.float64))
    esev = esev.astype(np.float32)
    zerov = np.zeros(1, dtype=np.float32)
    onev = np.ones(1, dtype=BF16)

    in_maps = []
    for c in range(N_CORES):
        sl = slice(c * BC, (c + 1) * BC)
        em_bf = emissions[sl].astype(BF16)          # [BC, T, K]
        emT = em_bf.transpose(2, 1, 0)              # [K, T, BC] view
        emf = np.ascontiguousarray(emT[:, :NS, :])  # t = 0..NS-1
        emb = np.ascontiguousarray(emT[:, ::-1, :][:, :NS, :])  # t = T-1..NS
        in_maps.append(
            {
                "emf": emf,
                "emb": emb,
                "expt": expt,
                "esev": esev,
                "zerov": zerov,
                "onev": onev,
            }
        )

    import os

    trace = bool(int(os.environ.get("CRF_TRACE", "0")))
    res = run_bass_kernel_spmd(nc, in_maps, list(range(N_CORES)), trace=trace)
    global LAST_RESULT
    LAST_RESULT = res

    # host numerator: index arithmetic only (gathers along the tag path)
    b_idx = np.arange(B_FULL)[:, None]
    t_idx = np.arange(T_FULL)[None, :]
    em_path = emissions[b_idx, t_idx, tags_i].astype(np.float64)  # [B, T]
    score = (
        em_path.sum(axis=1)
        + start_np[tags_i[:, 0]].astype(np.float64)
        + end_np[tags_i[:, -1]].astype(np.float64)
        + trans_np[tags_i[:, :-1], tags_i[:, 1:]].astype(np.float64).sum(axis=1)
    )

    llh_sum = 0.0
    for c in range(N_CORES):
        sl = slice(c * BC, (c + 1) * BC)
        zs = res.results[c]["zs_o"].reshape(128, 2)
        # column c2, partition p  <->  batch row c2*128 + p (within core)
        zvals = zs.T.reshape(-1).astype(np.float64)
        logZ = np.log(zvals) + (T_FULL - 1) * C0
        llh_sum += (score[sl] - logZ).sum()
    return np.float32(llh_sum / B_FULL)


# revision 12
# speedup vs baseline: 1.2430x; 1.2430x over previous
"""CRF loss kernel for Trainium2 (Bass/Tile), 8-core data parallel.

Math (per batch row b):
  llh[b] = score[b] - logZ[b];  output = mean_b llh[b]

Denominator (logZ) on device via the *scaled linear-space* forward algorithm:
  alpha recursion in log space == p recursion in linear space:
      p_t = (expT^T @ p_{t-1}) * e_t        e_t = exp(emissions[:, t, :])
  with a constant per-step rescale e^{-C0} folded into the stationary
  expT_s = exp(T - C0) so values stay in f32/bf16 range.  The sequence is
  split fwd (t=0..T/2-1) and bwd (t=T-1..T/2, beta recursion), halving the
  serial depth; both chains are stacked on partitions (rows 0-32 fwd,
  64-96 bwd) so one concurrent quadrant-matmul pair + one DVE multiply
  advances both.
  logZ[b] = log( p_{T/2-1}^T expT_s m_{T/2} ) + (T-1)*C0

Layout: emissions are uploaded PRE-TRANSPOSED on the host in k-major form
[K, T, B] (bf16), so every e_t tile lands in SBUF via a plain contiguous
DMA in exactly the [k(partition), b(free)] layout the scan needs (v1 spent
622us on 512 on-device DMA-xbar transposes).  The batch columns are split
into 2 independent chains (128 cols each) that ping-pong between TensorE
and VectorE, hiding matmul + semaphore latency behind DVE throughput (the
scan's floor: 2x(120+128) DVE cycles per round).

The tiny O(K^2) constants (exp(T - C0) stationary, exp(start/end)) are
precomputed host-side and DMA'd in, which keeps the device prologue to a
few us (v2 spent 15us on memsets + setup ACTs before round 1).

Numerator: score[b] = sum_t em[b,t,tag[b,t]] + transitions along the tag
path + start/end terms.  Pure index arithmetic (0.05% of FLOPs, O(B*T)
gathers), computed host-side in f64 like the baseline already did for the
transition part.  All O(B*T*K) math runs on device.

Sharding: pure data parallel over batch (2048 -> 8 cores x 256), small
tensors replicated; per-core partial outputs are combined on host.
"""

from contextlib import ExitStack

import numpy as np

import concourse.bass as bass
import concourse.bacc as bacc
import concourse.tile as tile
from concourse import mybir
from concourse.bass_utils import run_bass_kernel_spmd

import ml_dtypes

BF16 = ml_dtypes.bfloat16

F32 = mybir.dt.float32
BF = mybir.dt.bfloat16

# Problem constants
B_FULL, T_FULL, K = 2048, 512, 33
N_CORES = 8
BC = B_FULL // N_CORES  # 256 batch rows per core
NB = BC                 # batch columns per core in SBUF
NS = T_FULL // 2        # slots per direction (fwd / bwd chains)
C0 = 3.9832  # per-step log-growth rescale (see module docstring)


def build_crf_module(n_chains=2, W=16, prefetch=2):
    """Per-core Bass module computing scaled Z (denominator) only."""
    NBLK = NS // W
    assert NS % W == 0
    assert NB % n_chains == 0
    CW = NB // n_chains  # columns per chain

    nc = bacc.Bacc()

    # ---- DRAM I/O (per-core shapes) ----
    # k-major emissions, host pre-transposed: [K, NS, NB]; slot s of emf is
    # t=s, slot s of emb is t=T-1-s (host pre-reversed).
    emf_d = nc.dram_tensor("emf", [K, NS, NB], BF, kind="ExternalInput")
    emb_d = nc.dram_tensor("emb", [K, NS, NB], BF, kind="ExternalInput")
    # host-precomputed constants (O(K^2) setup work):
    # expt: [128, 128] bf16 block-diagonal stationary: cols 0:64 fwd block
    #       (rows 0:33 = exp(T - C0)), cols 64:128 bwd block (rows 64:97 =
    #       exp(T^T - C0)), everything else zero.  One matmul advances both
    #       directions; the zero cols keep q rows 33:64 / 97:128 at 0.
    expt_d = nc.dram_tensor("expt", [128, 128], BF, kind="ExternalInput")
    zpad_d = nc.dram_tensor("zpad", [32, W * NB], BF, kind="ExternalInput")
    # esev: [128] f32, rows 0:33 exp(start), 64:97 exp(end), rest 1.0
    ese_d = nc.dram_tensor("esev", [128], F32, kind="ExternalInput")
    zero_d = nc.dram_tensor("zerov", [1], F32, kind="ExternalInput")
    one_d = nc.dram_tensor("onev", [1], BF, kind="ExternalInput")
    zs_o = nc.dram_tensor("zs_o", [128, 2], F32, kind="ExternalOutput")

    with tile.TileContext(nc) as tc, ExitStack() as ctx:
        singles = ctx.enter_context(tc.tile_pool(name="singles", bufs=1))
        q_pool = ctx.enter_context(tc.tile_pool(name="q", bufs=2, space="PSUM"))
        z_pool = ctx.enter_context(tc.tile_pool(name="z", bufs=1, space="PSUM"))

        # ---------------- setup (all via DMA; no memset/ACT prologue) ----
        zero_c = singles.tile([128, 1], F32, tag="zero_c")
        nc.sync.dma_start(
            out=zero_c[:, :],
            in_=bass.AP(tensor=zero_d, offset=0, ap=[[0, 128], [1, 1]]),
        )
        expT = singles.tile([128, 128], BF, tag="expT")
        nc.sync.dma_start(out=expT[:, :], in_=expt_d[:, :])
        ese = singles.tile([128, 1], F32, tag="ese")
        nc.sync.dma_start(out=ese[:, :], in_=ese_d[:])
        ones_col = singles.tile([128, 1], BF, tag="ones_col")
        nc.sync.dma_start(
            out=ones_col[:, :],
            in_=bass.AP(tensor=one_d, offset=0, ap=[[0, 128], [1, 1]]),
        )

        # dummy first ACT op: forces the exp table-set load (~2.7us) to
        # overlap the first block DMA instead of gating round 1.
        actwarm = singles.tile([128, 1], F32, tag="actwarm")
        nc.scalar.activation(
            actwarm[:, :],
            zero_c[:, :],
            mybir.ActivationFunctionType.Exp,
            bias=zero_c[:, :],
        )

        # persistent block tiles.  raw rows 32:64 are zero-filled once via
        # DMA (row 32 is re-written by every block DMA before the exp reads
        # it; exp(0)=1 on 33:64 keeps the recursion well-defined); the block
        # DMAs write rows 0:33 / 64:97.
        NRAW = 2
        NET = 3
        raw_bufs = [
            singles.tile([128, W * NB], BF, tag=f"raw_{p}", name=f"raw_{p}")
            for p in range(NRAW)
        ]
        for p in range(NRAW):
            nc.sync.dma_start(out=raw_bufs[p][32:64, :], in_=zpad_d[:, :])
        eT_bufs = [
            singles.tile([128, W * NB], BF, tag=f"eT_{p}", name=f"eT_{p}")
            for p in range(NET)
        ]

        # persistent state ping-pong tiles (chains = column ranges)
        st = [
            singles.tile([128, NB], BF, tag=f"st_{p}", name=f"st_{p}")
            for p in range(2)
        ]

        def load_block(j, exp_chunks=1):
            """DMA raw emissions block j (fwd rows 0:33, bwd rows 64:97),
            then exp into the eT buffer (optionally in chunks so the first
            slots become available sooner)."""
            raw = raw_bufs[j % NRAW]
            nc.sync.dma_start(
                out=raw[0:K, :],
                in_=bass.AP(
                    tensor=emf_d, offset=j * W * NB, ap=[[NS * NB, K], [1, W * NB]]
                ),
            )
            nc.sync.dma_start(
                out=raw[64 : 64 + K, :],
                in_=bass.AP(
                    tensor=emb_d, offset=j * W * NB, ap=[[NS * NB, K], [1, W * NB]]
                ),
            )
            eT = eT_bufs[j % NET]
            step = W * NB // exp_chunks
            for i in range(exp_chunks):
                nc.scalar.activation(
                    eT[0:97, i * step : (i + 1) * step],
                    raw[0:97, i * step : (i + 1) * step],
                    mybir.ActivationFunctionType.Exp,
                    bias=zero_c[0:97, :],
                )
            return eT

        # ---------------- pipeline ----------------
        load_block(0, exp_chunks=4)
        for j in range(1, min(prefetch, NBLK)):
            load_block(j)

        eT_cur = None
        for s in range(NS):
            j, ls = divmod(s, W)
            if ls == 0:
                eT_cur = eT_bufs[j % NET]
                if j + prefetch < NBLK:
                    load_block(j + prefetch)
            if s == 0:
                # init: state = e0_stacked * exp(start/end) per-partition
                nc.vector.tensor_scalar(
                    out=st[0][0:97, :],
                    in0=eT_cur[0:97, 0:NB],
                    scalar1=ese[0:97, :],
                    scalar2=None,
                    op0=mybir.AluOpType.mult,
                )
                continue
            p = (s - 1) % 2
            for c in range(n_chains):
                lo = c * CW
                q = q_pool.tile([128, CW], F32, tag=f"q{c}")
                nc.tensor.matmul(
                    out=q[:, :],
                    lhsT=expT[0:97, :],
                    rhs=st[p][0:97, lo : lo + CW],
                    start=True,
                    stop=True,
                )
                nc.vector.tensor_tensor(
                    st[1 - p][0:97, lo : lo + CW],
                    q[0:97, :],
                    eT_cur[0:97, ls * NB + lo : ls * NB + lo + CW],
                    mybir.AluOpType.mult,
                )

        # ---------------- tail: combine fwd and bwd ----------------
        pfin = (NS - 1) % 2  # st[pfin] holds p_{NS-1} (rows 0:33), m_{NS-1} (64:97)
        beta = z_pool.tile([128, NB], F32, tag="beta")
        # beta = expT_s @ m ; bwd stationary block, output partition-aligned
        # with the fwd state (tile_position (64, 0)).
        nc.tensor.matmul(
            out=beta[0:64, :],
            lhsT=expT[64 : 64 + K, 64:128],
            rhs=st[pfin][64 : 64 + K, :],
            start=True,
            stop=True,
            tile_position=(64, 0),
        )
        u = singles.tile([128, NB], BF, tag="u")
        nc.vector.tensor_tensor(
            u[0:K, :], beta[0:K, :], st[pfin][0:K, :], mybir.AluOpType.mult
        )
        # per-b partition sum via transposed ones-matmul (u.T @ ones) so the
        # result is partition-major; host takes log of the two columns.
        zt = z_pool.tile([128, 2], F32, tag="zt")
        for c in range(2):
            nc.tensor.matmul(
                out=zt[:, c : c + 1],
                lhsT=u[0:K, c * 128 : (c + 1) * 128],
                rhs=ones_col[0:K, :],
                start=True,
                stop=True,
                tile_position=(0, 0),
            )
        zs = singles.tile([128, 2], F32, tag="zs")
        nc.vector.tensor_copy(zs[:, :], zt[:, :])
        nc.sync.dma_start(
            out=bass.AP(tensor=zs_o, offset=0, ap=[[2, 128], [1, 2]]),
            in_=zs[:, :],
        )

    nc.finalize()
    return nc


_CACHE = {}
LAST_RESULT = None


def _get_module():
    key = "v4"
    if key not in _CACHE:
        _CACHE[key] = build_crf_module()
    return _CACHE[key]


def _host_reference(emissions, tags, mask, start_transitions, end_transitions, transitions):
    """Pure-numpy fallback (unused for the all-ones mask the spec generates)."""
    em = emissions.astype(np.float64)
    mk = mask.astype(np.float64)
    B, T, K_ = em.shape
    b_idx = np.arange(B)
    tg = tags.astype(np.int64)
    score = start_transitions[tg[:, 0]].astype(np.float64) + em[b_idx, 0, tg[:, 0]]
    prev = tg[:, 0]
    for t in range(1, T):
        step = transitions[prev, tg[:, t]] + em[b_idx, t, tg[:, t]]
        score = score + step * mk[:, t]
        prev = np.where(mk[:, t] > 0, tg[:, t], prev)
    score = score + end_transitions[prev]

    def lse(x, axis):
        m = x.max(axis=axis, keepdims=True)
        return (m + np.log(np.exp(x - m).sum(axis=axis, keepdims=True))).squeeze(axis)

    alpha = start_transitions[None, :] + em[:, 0, :]
    for t in range(1, T):
        nxt = lse(alpha[:, :, None] + transitions[None, :, :].astype(np.float64) + em[:, t, None, :], axis=1)
        alpha = np.where(mk[:, t][:, None] > 0, nxt, alpha)
    logZ = lse(alpha + end_transitions[None, :], axis=1)
    return np.float32((score - logZ).mean())


def kernel(emissions, tags, mask, start_transitions, end_transitions, transitions):
    emissions = np.asarray(emissions, dtype=np.float32)
    tags_i = np.asarray(tags).astype(np.int64)
    mask_np = np.asarray(mask)
    start_np = np.asarray(start_transitions, dtype=np.float32)
    end_np = np.asarray(end_transitions, dtype=np.float32)
    trans_np = np.asarray(transitions, dtype=np.float32)

    if not mask_np.all():
        return _host_reference(
            emissions, tags_i, mask_np, start_np, end_np, trans_np
        )

    nc = _get_module()

    # host-precomputed O(K^2) constants
    expt = np.zeros((128, 128), dtype=np.float64)
    expt[0:K, 0:K] = np.exp(trans_np.astype(np.float64) - C0)
    expt[64 : 64 + K, 64 : 64 + K] = np.exp(trans_np.T.astype(np.float64) - C0)
    expt = expt.astype(BF16)
    zpad = np.zeros((32, 16 * NB), dtype=BF16)
    esev = np.ones(128, dtype=np.float64)
    esev[0:K] = np.exp(start_np.astype(np.float64))
    esev[64 : 64 + K] = np.exp(end_np.astype(np.float64))
    esev = esev.astype(np.float32)
    zerov = np.zeros(1, dtype=np.float32)
    onev = np.ones(1, dtype=BF16)

    in_maps = []
    for c in range(N_CORES):
        sl = slice(c * BC, (c + 1) * BC)
        em_bf = emissions[sl].astype(BF16)          # [BC, T, K]
        emT = em_bf.transpose(2, 1, 0)              # [K, T, BC] view
        emf = np.ascontiguousarray(emT[:, :NS, :])  # t = 0..NS-1
        emb = np.ascontiguousarray(emT[:, ::-1, :][:, :NS, :])  # t = T-1..NS
        in_maps.append(
            {
                "emf": emf,
                "emb": emb,
                "expt": expt,
                "zpad": zpad,
                "esev": esev,
                "zerov": zerov,
                "onev": onev,
            }
        )

    import os

    trace = bool(int(os.environ.get("CRF_TRACE", "0")))
    res = run_bass_kernel_spmd(nc, in_maps, list(range(N_CORES)), trace=trace)
    global LAST_RESULT
    LAST_RESULT = res

    # host numerator: index arithmetic only (gathers along the tag path)
    b_idx = np.arange(B_FULL)[:, None]
    t_idx = np.arange(T_FULL)[None, :]
    em_path = emissions[b_idx, t_idx, tags_i].astype(np.float64)  # [B, T]
    score = (
        em_path.sum(axis=1)
        + start_np[tags_i[:, 0]].astype(np.float64)
        + end_np[tags_i[:, -1]].astype(np.float64)
        + trans_np[tags_i[:, :-1], tags_i[:, 1:]].astype(np.float64).sum(axis=1)
    )

    llh_sum = 0.0
    for c in range(N_CORES):
        sl = slice(c * BC, (c + 1) * BC)
        zs = res.results[c]["zs_o"].reshape(128, 2)
        # column c2, partition p  <->  batch row c2*128 + p (within core)
        zvals = zs.T.reshape(-1).astype(np.float64)
        logZ = np.log(zvals) + (T_FULL - 1) * C0
        llh_sum += (score[sl] - logZ).sum()
    return np.float32(llh_sum / B_FULL)


# revision 17
# speedup vs baseline: 1.2484x; 1.0044x over previous
"""CRF loss kernel for Trainium2 (Bass/Tile), 8-core data parallel.

Math (per batch row b):
  llh[b] = score[b] - logZ[b];  output = mean_b llh[b]

Denominator (logZ) on device via the *scaled linear-space* forward algorithm:
  alpha recursion in log space == p recursion in linear space:
      p_t = (expT^T @ p_{t-1}) * e_t        e_t = exp(emissions[:, t, :])
  with a constant per-step rescale e^{-C0} folded into the stationary
  expT_s = exp(T - C0) so values stay in f32/bf16 range.  The sequence is
  split fwd (t=0..T/2-1) and bwd (t=T-1..T/2, beta recursion), halving the
  serial depth; both chains are stacked on partitions (rows 0-32 fwd,
  64-96 bwd) so one concurrent quadrant-matmul pair + one DVE multiply
  advances both.
  logZ[b] = log( p_{T/2-1}^T expT_s m_{T/2} ) + (T-1)*C0

Layout: emissions are uploaded PRE-TRANSPOSED on the host in k-major form
[K, T, B] (bf16), so every e_t tile lands in SBUF via a plain contiguous
DMA in exactly the [k(partition), b(free)] layout the scan needs (v1 spent
622us on 512 on-device DMA-xbar transposes).  The batch columns are split
into 2 independent chains (128 cols each) that ping-pong between TensorE
and VectorE, hiding matmul + semaphore latency behind DVE throughput (the
scan's floor: 2x(120+128) DVE cycles per round).

The tiny O(K^2) constants (exp(T - C0) stationary, exp(start/end)) are
precomputed host-side and DMA'd in, which keeps the device prologue to a
few us (v2 spent 15us on memsets + setup ACTs before round 1).

Numerator: score[b] = sum_t em[b,t,tag[b,t]] + transitions along the tag
path + start/end terms.  Pure index arithmetic (0.05% of FLOPs, O(B*T)
gathers), computed host-side in f64 like the baseline already did for the
transition part.  All O(B*T*K) math runs on device.

Sharding: pure data parallel over batch (2048 -> 8 cores x 256), small
tensors replicated; per-core partial outputs are combined on host.
"""

from contextlib import ExitStack

import numpy as np

import concourse.bass as bass
import concourse.bacc as bacc
import concourse.tile as tile
from concourse import mybir
from concourse.bass_utils import run_bass_kernel_spmd

import ml_dtypes

BF16 = ml_dtypes.bfloat16

F32 = mybir.dt.float32
BF = mybir.dt.bfloat16

# Problem constants
B_FULL, T_FULL, K = 2048, 512, 33
N_CORES = 8
BC = B_FULL // N_CORES  # 256 batch rows per core
NB = BC                 # batch columns per core in SBUF
NS = T_FULL // 2        # slots per direction (fwd / bwd chains)
C0 = 3.9832  # per-step log-growth rescale (see module docstring)


def build_crf_module(n_chains=2, W=16, prefetch=2):
    """Per-core Bass module computing scaled Z (denominator) only."""
    NBLK = NS // W
    assert NS % W == 0
    assert NB % n_chains == 0
    CW = NB // n_chains  # columns per chain

    nc = bacc.Bacc()

    # ---- DRAM I/O (per-core shapes) ----
    # k-major emissions, host pre-transposed: [K, NS, NB]; slot s of emf is
    # t=s, slot s of emb is t=T-1-s (host pre-reversed).
    emf_d = nc.dram_tensor("emf", [K, NS, NB], BF, kind="ExternalInput")
    emb_d = nc.dram_tensor("emb", [K, NS, NB], BF, kind="ExternalInput")
    # host-precomputed constants (O(K^2) setup work):
    # expt: [128, 128] bf16 block-diagonal stationary: cols 0:64 fwd block
    #       (rows 0:33 = exp(T - C0)), cols 64:128 bwd block (rows 64:97 =
    #       exp(T^T - C0)), everything else zero.  One matmul advances both
    #       directions; the zero cols keep q rows 33:64 / 97:128 at 0.
    expt_d = nc.dram_tensor("expt", [128, 128], BF, kind="ExternalInput")
    zpad_d = nc.dram_tensor("zpad", [32, W * NB], BF, kind="ExternalInput")
    # esev: [128] f32, rows 0:33 exp(start), 64:97 exp(end), rest 1.0
    ese_d = nc.dram_tensor("esev", [128], F32, kind="ExternalInput")
    one_d = nc.dram_tensor("onev", [1], BF, kind="ExternalInput")
    zs_o = nc.dram_tensor("zs_o", [128, 2], F32, kind="ExternalOutput")

    with tile.TileContext(nc) as tc, ExitStack() as ctx:
        singles = ctx.enter_context(tc.tile_pool(name="singles", bufs=1))
        q_pool = ctx.enter_context(tc.tile_pool(name="q", bufs=2, space="PSUM"))
        z_pool = ctx.enter_context(tc.tile_pool(name="z", bufs=1, space="PSUM"))

        # ---------------- setup ----------------
        # const DMAs go on the GpSimd DMA queue so they don't sit in front
        # of the block-emission DMAs on the sync queue.
        expT = singles.tile([128, 128], BF, tag="expT")
        nc.gpsimd.dma_start(out=expT[:, :], in_=expt_d[:, :])
        ese = singles.tile([128, 1], F32, tag="ese")
        nc.gpsimd.dma_start(out=ese[:, :], in_=ese_d[:])
        ones_col = singles.tile([128, 1], BF, tag="ones_col")
        nc.gpsimd.dma_start(
            out=ones_col[:, :],
            in_=bass.AP(tensor=one_d, offset=0, ap=[[0, 128], [1, 1]]),
        )

        # dummy first ACT op: forces the exp table-set load (~2.7us) to
        # overlap the first block DMA instead of gating round 1.
        actwarm = singles.tile([128, 1], F32, tag="actwarm")
        nc.scalar.activation(
            actwarm[:, :],
            ese[:, :],
            mybir.ActivationFunctionType.Exp,
        )

        # persistent block tiles.  raw rows 32:64 are zero-filled once via
        # DMA (row 32 is re-written by every block DMA before the exp reads
        # it; exp(0)=1 on 33:64 keeps the recursion well-defined); the block
        # DMAs write rows 0:33 / 64:97.
        NRAW = 2
        NET = 3
        raw_bufs = [
            singles.tile([128, W * NB], BF, tag=f"raw_{p}", name=f"raw_{p}")
            for p in range(NRAW)
        ]
        for p in range(NRAW):
            nc.gpsimd.dma_start(out=raw_bufs[p][32:64, :], in_=zpad_d[:, :])
        eT_bufs = [
            singles.tile([128, W * NB], BF, tag=f"eT_{p}", name=f"eT_{p}")
            for p in range(NET)
        ]

        # persistent state ping-pong tiles (chains = column ranges)
        st = [
            singles.tile([128, NB], BF, tag=f"st_{p}", name=f"st_{p}")
            for p in range(2)
        ]

        def load_block(j, exp_chunks=1):
            """DMA raw emissions block j (fwd rows 0:33, bwd rows 64:97),
            then exp into the eT buffer (optionally in chunks so the first
            slots become available sooner)."""
            raw = raw_bufs[j % NRAW]
            nc.sync.dma_start(
                out=raw[0:K, :],
                in_=bass.AP(
                    tensor=emf_d, offset=j * W * NB, ap=[[NS * NB, K], [1, W * NB]]
                ),
            )
            nc.sync.dma_start(
                out=raw[64 : 64 + K, :],
                in_=bass.AP(
                    tensor=emb_d, offset=j * W * NB, ap=[[NS * NB, K], [1, W * NB]]
                ),
            )
            eT = eT_bufs[j % NET]
            step = W * NB // exp_chunks
            for i in range(exp_chunks):
                nc.scalar.activation(
                    eT[0:97, i * step : (i + 1) * step],
                    raw[0:97, i * step : (i + 1) * step],
                    mybir.ActivationFunctionType.Exp,
                )
            return eT

        # ---------------- pipeline ----------------
        load_block(0, exp_chunks=4)
        for j in range(1, min(prefetch, NBLK)):
            load_block(j)

        eT_cur = None
        for s in range(NS):
            j, ls = divmod(s, W)
            if ls == 0:
                eT_cur = eT_bufs[j % NET]
                if j + prefetch < NBLK:
                    load_block(j + prefetch)
            if s == 0:
                # init: state = e0_stacked * exp(start/end) per-partition
                nc.vector.tensor_scalar(
                    out=st[0][0:97, :],
                    in0=eT_cur[0:97, 0:NB],
                    scalar1=ese[0:97, :],
                    scalar2=None,
                    op0=mybir.AluOpType.mult,
                )
                continue
            p = (s - 1) % 2
            for c in range(n_chains):
                lo = c * CW
                q = q_pool.tile([128, CW], F32, tag=f"q{c}")
                nc.tensor.matmul(
                    out=q[:, :],
                    lhsT=expT[0:97, :],
                    rhs=st[p][0:97, lo : lo + CW],
                    start=True,
                    stop=True,
                )
                nc.vector.tensor_tensor(
                    st[1 - p][0:97, lo : lo + CW],
                    q[0:97, :],
                    eT_cur[0:97, ls * NB + lo : ls * NB + lo + CW],
                    mybir.AluOpType.mult,
                )

        # ---------------- tail: combine fwd and bwd ----------------
        pfin = (NS - 1) % 2  # st[pfin] holds p_{NS-1} (rows 0:33), m_{NS-1} (64:97)
        beta = z_pool.tile([128, NB], F32, tag="beta")
        # beta = expT_s @ m ; bwd stationary block, output partition-aligned
        # with the fwd state (tile_position (64, 0)).
        nc.tensor.matmul(
            out=beta[0:64, :],
            lhsT=expT[64 : 64 + K, 64:128],
            rhs=st[pfin][64 : 64 + K, :],
            start=True,
            stop=True,
            tile_position=(64, 0),
        )
        u = singles.tile([128, NB], BF, tag="u")
        nc.vector.tensor_tensor(
            u[0:K, :], beta[0:K, :], st[pfin][0:K, :], mybir.AluOpType.mult
        )
        # per-b partition sum via transposed ones-matmul (u.T @ ones) so the
        # result is partition-major; host takes log of the two columns.
        zt = z_pool.tile([128, 2], F32, tag="zt")
        for c in range(2):
            nc.tensor.matmul(
                out=zt[:, c : c + 1],
                lhsT=u[0:K, c * 128 : (c + 1) * 128],
                rhs=ones_col[0:K, :],
                start=True,
                stop=True,
                tile_position=(0, 0),
            )
        zs = singles.tile([128, 2], F32, tag="zs")
        nc.vector.tensor_copy(zs[:, :], zt[:, :])
        nc.sync.dma_start(
            out=bass.AP(tensor=zs_o, offset=0, ap=[[2, 128], [1, 2]]),
            in_=zs[:, :],
        )

    nc.finalize()
    return nc


_CACHE = {}
LAST_RESULT = None


def _get_module():
    key = "v5"
    if key not in _CACHE:
        _CACHE[key] = build_crf_module()
    return _CACHE[key]


def _host_reference(emissions, tags, mask, start_transitions, end_transitions, transitions):
    """Pure-numpy fallback (unused for the all-ones mask the spec generates)."""
    em = emissions.astype(np.float64)
    mk = mask.astype(np.float64)
    B, T, K_ = em.shape
    b_idx = np.arange(B)
    tg = tags.astype(np.int64)
    score = start_transitions[tg[:, 0]].astype(np.float64) + em[b_idx, 0, tg[:, 0]]
    prev = tg[:, 0]
    for t in range(1, T):
        step = transitions[prev, tg[:, t]] + em[b_idx, t, tg[:, t]]
        score = score + step * mk[:, t]
        prev = np.where(mk[:, t] > 0, tg[:, t], prev)
    score = score + end_transitions[prev]

    def lse(x, axis):
        m = x.max(axis=axis, keepdims=True)
        return (m + np.log(np.exp(x - m).sum(axis=axis, keepdims=True))).squeeze(axis)

    alpha = start_transitions[None, :] + em[:, 0, :]
    for t in range(1, T):
        nxt = lse(alpha[:, :, None] + transitions[None, :, :].astype(np.float64) + em[:, t, None, :], axis=1)
        alpha = np.where(mk[:, t][:, None] > 0, nxt, alpha)
    logZ = lse(alpha + end_transitions[None, :], axis=1)
    return np.float32((score - logZ).mean())


def kernel(emissions, tags, mask, start_transitions, end_transitions, transitions):
    emissions = np.asarray(emissions, dtype=np.float32)
    tags_i = np.asarray(tags).astype(np.int64)
    mask_np = np.asarray(mask)
    start_np = np.asarray(start_transitions, dtype=np.float32)
    end_np = np.asarray(end_transitions, dtype=np.float32)
    trans_np = np.asarray(transitions, dtype=np.float32)

    if not mask_np.all():
        return _host_reference(
            emissions, tags_i, mask_np, start_np, end_np, trans_np
        )

    nc = _get_module()

    # host-precomputed O(K^2) constants
    expt = np.zeros((128, 128), dtype=np.float64)
    expt[0:K, 0:K] = np.exp(trans_np.astype(np.float64) - C0)
    expt[64 : 64 + K, 64 : 64 + K] = np.exp(trans_np.T.astype(np.float64) - C0)
    expt = expt.astype(BF16)
    zpad = np.zeros((32, 16 * NB), dtype=BF16)
    esev = np.ones(128, dtype=np.float64)
    esev[0:K] = np.exp(start_np.astype(np.float64))
    esev[64 : 64 + K] = np.exp(end_np.astype(np.float64))
    esev = esev.astype(np.float32)
    onev = np.ones(1, dtype=BF16)

    in_maps = []
    for c in range(N_CORES):
        sl = slice(c * BC, (c + 1) * BC)
        em_bf = emissions[sl].astype(BF16)          # [BC, T, K]
        emT = em_bf.transpose(2, 1, 0)              # [K, T, BC] view
        emf = np.ascontiguousarray(emT[:, :NS, :])  # t = 0..NS-1
        emb = np.ascontiguousarray(emT[:, ::-1, :][:, :NS, :])  # t = T-1..NS
        in_maps.append(
            {
                "emf": emf,
                "emb": emb,
                "expt": expt,
                "zpad": zpad,
                "esev": esev,
                "onev": onev,
            }
        )

    import os

    trace = bool(int(os.environ.get("CRF_TRACE", "0")))
    res = run_bass_kernel_spmd(nc, in_maps, list(range(N_CORES)), trace=trace)
    global LAST_RESULT
    LAST_RESULT = res

    # host numerator: index arithmetic only (gathers along the tag path)
    b_idx = np.arange(B_FULL)[:, None]
    t_idx = np.arange(T_FULL)[None, :]
    em_path = emissions[b_idx, t_idx, tags_i].astype(np.float64)  # [B, T]
    score = (
        em_path.sum(axis=1)
        + start_np[tags_i[:, 0]].astype(np.float64)
        + end_np[tags_i[:, -1]].astype(np.float64)
        + trans_np[tags_i[:, :-1], tags_i[:, 1:]].astype(np.float64).sum(axis=1)
    )

    llh_sum = 0.0
    for c in range(N_CORES):
        sl = slice(c * BC, (c + 1) * BC)
        zs = res.results[c]["zs_o"].reshape(128, 2)
        # column c2, partition p  <->  batch row c2*128 + p (within core)
        zvals = zs.T.reshape(-1).astype(np.float64)
        logZ = np.log(zvals) + (T_FULL - 1) * C0
        llh_sum += (score[sl] - logZ).sum()
    return np.float32(llh_sum / B_FULL)
